# revision 1
# baseline (speedup 1.0000x reference)
"""Trainium2 Bass kernel for FMGCNCell (adaptive-graph GRU cell).

Sharding: node-parallel. Each of the 8 cores owns N/8 = 250 output nodes,
with the full batch B=64. The NxN adaptive support is built column-sliced
per core (symmetric trick), the two graph convs contract over all 2000
nodes with per-core output-node slices, and the per-node generated-weight
application runs with the full batch on the PE. The only cross-core
exchange is an AllGather of z*state between the two graph convs.
"""

import math
from contextlib import ExitStack

import numpy as np
import ml_dtypes

import concourse.bass as bass
import concourse.bacc as bacc
import concourse.tile as tile
from concourse import mybir
from concourse.bass import ds, ts
from concourse.bass_utils import run_bass_kernel_spmd
from concourse.masks import make_identity

F32 = mybir.dt.float32
BF16 = mybir.dt.bfloat16
AF = mybir.ActivationFunctionType
ALU = mybir.AluOpType

# Problem constants
B = 64
DIN = 2
H = 64
E = 16
K = 2
CAT = DIN + H            # 66
KI0 = CAT + 1            # 67 rows: 66 inputs + 1 bias row
KIALL = 2 * CAT + 1      # 133 rows in the generated weight tensor
OG = 2 * H               # 128 gate outputs (z|r)
OU = H                   # 64 update outputs
BC = B * CAT             # 4224 = 33 * 128


def _chunks(total, size):
    out = []
    off = 0
    while off < total:
        out.append((off, min(size, total - off)))
        off += size
    return out


def build_nc(N=2000, n_cores=8, nblk=25):
    """Builds the SPMD program (identical on all cores; per-core data differs)."""
    NOWN = N // n_cores
    mch = _chunks(N, 128)          # chunks of the contraction/node axis
    bcch = _chunks(BC, 128)        # chunks of the (b,c) axis: 33 x 128
    MC = len(mch)

    nc = bacc.Bacc("TRN2", target_bir_lowering=False, debug=False,
                   num_devices=n_cores)

    # ---- external inputs (per-core data supplied by kernel()) ----
    x1_d = nc.dram_tensor("x1_in", [N, BC], BF16, kind="ExternalInput")
    xt_d = nc.dram_tensor("xt_in", [KI0, B * NOWN], BF16, kind="ExternalInput")
    state_own_d = nc.dram_tensor("state_own", [B, NOWN, H], F32, kind="ExternalInput")
    eT_all_d = nc.dram_tensor("eT_all", [E, N], F32, kind="ExternalInput")
    eT_own_d = nc.dram_tensor("eT_own", [E, NOWN], F32, kind="ExternalInput")
    eT_own_bf_d = nc.dram_tensor("eT_own_bf", [E, NOWN], BF16, kind="ExternalInput")
    # weight pools, host-packed: [E, 133, O] with bias row at index 66
    wpg_d = nc.dram_tensor("wpg", [E, KIALL, OG], BF16, kind="ExternalInput")
    wpu_d = nc.dram_tensor("wpu", [E, KIALL, OU], BF16, kind="ExternalInput")

    out_d = nc.dram_tensor("out", [B, NOWN, H], F32, kind="ExternalOutput")

    # ---- internal DRAM ----
    xg1_d = nc.dram_tensor("xg1_d", [BC, NOWN], BF16)
    xg2_d = nc.dram_tensor("xg2_d", [BC, NOWN], BF16)
    wg_d = nc.dram_tensor("wg_d", [NOWN, KIALL, OG], BF16)
    wu_d = nc.dram_tensor("wu_d", [NOWN, KIALL, OU], BF16)
    zs_own_d = nc.dram_tensor("zs_own_d", [NOWN, B, H], BF16)
    r_d = nc.dram_tensor("r_d", [NOWN, B, H], BF16)
    if n_cores > 1:
        zs_all_d = nc.dram_tensor("zs_all_d", [N, B, H], BF16, addr_space="Shared")
    else:
        zs_all_d = nc.dram_tensor("zs_all_d", [N, B, H], BF16)

    with tile.TileContext(nc) as tc:
        with ExitStack() as root:
            # ---------- long-lived tiles ----------
            persist = root.enter_context(tc.tile_pool(name="persist", bufs=1))
            # M[m, n_own] per m-chunk, bf16: [128, MC*NOWN]
            M_sb = persist.tile([128, MC * NOWN], BF16)
            # transposed inputs for the k=0 slab (+ ones row at 66):
            # XT[c', b*NOWN + n]
            XT = persist.tile([KI0, B * NOWN], BF16)
            rinv_bc = persist.tile([128, NOWN], F32)
            # keep the x columns of X (tiny) for rebuilding X2:
            # per m-chunk, cols (b, 0:2) -> [128, MC * B * DIN]
            xcols = persist.tile([128, MC * B * DIN], BF16)

            # ---------- phase 1: S -> M -> rowsum; X build; XT build ----------
            with ExitStack() as p1:
                sm_pool = p1.enter_context(tc.tile_pool(name="sm", bufs=3))
                sm_psum = p1.enter_context(tc.tile_pool(name="sm_ps", bufs=2, space="PSUM"))
                eT_pool = p1.enter_context(tc.tile_pool(name="eT", bufs=1))

                eT_all = eT_pool.tile([E, N], F32)
                nc.sync.dma_start(eT_all[:], eT_all_d[:])
                eT_own = eT_pool.tile([E, NOWN], F32)
                nc.sync.dma_start(eT_own[:], eT_own_d[:])

                for j, (m0, mp) in enumerate(mch):
                    ps = sm_psum.tile([128, NOWN], F32)
                    nc.tensor.matmul(ps[:mp, :], eT_all[:, m0:m0 + mp],
                                     eT_own[:, :], start=True, stop=True)
                    ex = sm_pool.tile([128, NOWN], F32)
                    nc.scalar.activation(ex[:mp, :], ps[:mp, :], AF.Exp)
                    # M = max(exp(S), 1)  (== exp(relu(S)))
                    nc.vector.tensor_scalar_max(M_sb[:mp, ts(j, NOWN)], ex[:mp, :],
                                                1.0)

                # rowsum via ones-matmul over m (symmetric A)
                ones_pool = p1.enter_context(tc.tile_pool(name="ones", bufs=1))
                ones = ones_pool.tile([128, 1], BF16)
                nc.vector.memset(ones[:], 1.0)
                rs_ps = sm_psum.tile([1, NOWN], F32)
                for j, (m0, mp) in enumerate(mch):
                    nc.tensor.matmul(rs_ps[:, :], ones[:mp, :],
                                     M_sb[:mp, ts(j, NOWN)],
                                     start=(j == 0), stop=(j == MC - 1))
                rinv = sm_pool.tile([1, NOWN], F32)
                nc.vector.reciprocal(rinv[:, :], rs_ps[:, :])
                nc.gpsimd.partition_broadcast(rinv_bc[:], rinv[:, :])

            n0_own = 0  # own node range start differs per core ONLY in data
            # ---------- phase 2: X build (SBUF resident), XT, A@X1 ----------
            with ExitStack() as p2:
                x_pool = p2.enter_context(tc.tile_pool(name="xsb", bufs=1))
                X_sb = x_pool.tile([128, MC * BC], BF16)
                for j, (m0, mp) in enumerate(mch):
                    nc.sync.dma_start(X_sb[:mp, j * BC:(j + 1) * BC],
                                      x1_d[m0:m0 + mp, :])
                    # keep the x columns (b, 64:66) for rebuilding X2
                    nc.scalar.copy(
                        xcols[:mp, ts(j, B * DIN)]
                        .rearrange("m (b c) -> m b c", c=DIN),
                        X_sb[:mp, j * BC:(j + 1) * BC]
                        .rearrange("m (b c) -> m b c", c=CAT)[:, :, H:])
                nc.sync.dma_start(XT[:], xt_d[:])
                # A @ X1 -> xg1_d
                ax_psum = p2.enter_context(tc.tile_pool(name="ax_ps", bufs=3, space="PSUM"))
                ax_pool = p2.enter_context(tc.tile_pool(name="ax", bufs=3))
                for q, (c0, cp) in enumerate(bcch):
                    ps = ax_psum.tile([128, NOWN], F32, tag="ax_ps")
                    for j, (m0, mp) in enumerate(mch):
                        nc.tensor.matmul(ps[:cp, :],
                                         X_sb[:mp, j * BC + c0: j * BC + c0 + cp],
                                         M_sb[:mp, ts(j, NOWN)],
                                         start=(j == 0), stop=(j == MC - 1))
                    ev = ax_pool.tile([128, NOWN], BF16, tag="ax_ev")
                    nc.vector.tensor_tensor(ev[:cp, :], ps[:cp, :],
                                            rinv_bc[:cp, :], ALU.mult)
                    nc.sync.dma_start(xg1_d[c0:c0 + cp, :], ev[:cp, :])

            # ---------- phase 3: W-gen (gate+update) -> DRAM ----------
            nch = _chunks(NOWN, 128)
            with ExitStack() as p3:
                wp_pool = p3.enter_context(tc.tile_pool(name="wp", bufs=1))
                eT_own_bf = wp_pool.tile([E, NOWN], BF16)
                nc.sync.dma_start(eT_own_bf[:], eT_own_bf_d[:])
                wg_sb = wp_pool.tile([E, KIALL * OG], BF16)
                nc.sync.dma_start(wg_sb[:], wpg_d[:].rearrange("e k o -> e (k o)"))
                wu_sb = wp_pool.tile([E, KIALL * OU], BF16)
                nc.sync.dma_start(wu_sb[:], wpu_d[:].rearrange("e k o -> e (k o)"))
                wgen_ps = p3.enter_context(tc.tile_pool(name="wg_ps", bufs=3, space="PSUM"))
                wgen_pool = p3.enter_context(tc.tile_pool(name="wg_ev", bufs=3))
                for (wsb, wdram, O) in ((wg_sb, wg_d, OG), (wu_sb, wu_d, OU)):
                    KO = KIALL * O
                    for (nn0, np_) in nch:
                        for (f0, fp) in _chunks(KO, 512):
                            ps = wgen_ps.tile([128, 512], F32, tag="wg_ps")
                            nc.tensor.matmul(ps[:np_, :fp],
                                             eT_own_bf[:, nn0:nn0 + np_],
                                             wsb[:, f0:f0 + fp],
                                             start=True, stop=True)
                            ev = wgen_pool.tile([128, 512], BF16, tag="wg_ev")
                            nc.scalar.copy(ev[:np_, :fp], ps[:np_, :fp])
                            nc.sync.dma_start(
                                wdram[:].rearrange("n k o -> n (k o)")
                                [nn0:nn0 + np_, f0:f0 + fp],
                                ev[:np_, :fp])

            # ---------- phase 4: apply gate; z*state; write zs_own ----------
            nblocks = _chunks(NOWN, nblk)
            with ExitStack() as p4:
                xg1T_pool = p4.enter_context(tc.tile_pool(name="xg1T", bufs=1))
                xg1T = xg1T_pool.tile([CAT, B * NOWN], BF16)
                # reload: dram[(b,c), n] -> [c, (b, n)]
                nc.sync.dma_start(
                    xg1T[:].rearrange("c (b n) -> c b n", b=B),
                    xg1_d[:].rearrange("(b c) n -> c b n", b=B))
                ap_w = p4.enter_context(tc.tile_pool(name="ap_w", bufs=2))
                ap_ps = p4.enter_context(tc.tile_pool(name="ap_ps", bufs=4, space="PSUM"))
                ap_ev = p4.enter_context(tc.tile_pool(name="ap_ev", bufs=3))
                st_w = p4.enter_context(tc.tile_pool(name="st_w", bufs=2))
                zt_ps = p4.enter_context(tc.tile_pool(name="zt_ps", bufs=3, space="PSUM"))
                id_pool4 = p4.enter_context(tc.tile_pool(name="id4", bufs=1))
                ident = id_pool4.tile([128, 128], BF16)
                make_identity(nc, ident[:])
                for (nb0, nbp) in nblocks:
                    zs_blk = st_w.tile([B, nblk * H], BF16, tag="zs_blk")
                    r_blk = st_w.tile([B, nblk * H], BF16, tag="r_blk")
                    w0 = ap_w.tile([KI0, nblk * OG], BF16, tag="w0")
                    nc.sync.dma_start(
                        w0[:, :nbp * OG].rearrange("k (n o) -> k n o", o=OG),
                        wg_d[nb0:nb0 + nbp, :KI0, :].rearrange("n k o -> k n o"))
                    w1 = ap_w.tile([CAT, nblk * OG], BF16, tag="w1")
                    nc.sync.dma_start(
                        w1[:, :nbp * OG].rearrange("k (n o) -> k n o", o=OG),
                        wg_d[nb0:nb0 + nbp, KI0:, :].rearrange("n k o -> k n o"))
                    stw = st_w.tile([B, nblk * H], F32, tag="stw")
                    nc.sync.dma_start(
                        stw[:, :nbp * H],
                        state_own_d[:, nb0:nb0 + nbp, :]
                        .rearrange("b n h -> b (n h)"))
                    for nl in range(nbp):
                        n = nb0 + nl
                        ps = ap_ps.tile([B, OG], F32, tag="ap_ps")
                        nc.tensor.matmul(
                            ps[:, :],
                            XT[:, n::NOWN][:, :B],
                            w0[:, nl * OG:(nl + 1) * OG],
                            start=True, stop=False)
                        nc.tensor.matmul(
                            ps[:, :],
                            xg1T[:, n::NOWN][:, :B],
                            w1[:, nl * OG:(nl + 1) * OG],
                            start=False, stop=True)
                        # z, r
                        zt = ap_ev.tile([B, H], F32, tag="zt")
                        nc.scalar.activation(zt[:, :], ps[:, :H], AF.Sigmoid)
                        nc.scalar.activation(
                            r_blk[:, nl * H:(nl + 1) * H], ps[:, H:], AF.Sigmoid)
                        # z * state
                        zs = zs_blk[:, nl * H:(nl + 1) * H]
                        nc.vector.tensor_tensor(
                            zs[:, :], zt[:, :], stw[:, nl * H:(nl + 1) * H],
                            ALU.mult)
                        # transpose z*state into XT rows 0:64 (cand_in.T)
                        zs_t = zt_ps.tile([H, B], BF16, tag="zs_t")
                        nc.tensor.transpose(zs_t[:, :], zs[:, :], ident[:B, :B])
                        nc.vector.tensor_copy(
                            XT[:H, n::NOWN], zs_t[:, :])
                    nc.sync.dma_start(
                        zs_own_d[nb0:nb0 + nbp, :, :].rearrange("n b h -> b n h"),
                        zs_blk[:, :nbp * H].rearrange("b (n h) -> b n h", h=H))
                    nc.sync.dma_start(
                        r_d[nb0:nb0 + nbp, :, :].rearrange("n b h -> b n h"),
                        r_blk[:, :nbp * H].rearrange("b (n h) -> b n h", h=H))

            # ---------- phase 5: allgather z*state ----------
            if n_cores > 1:
                nc.gpsimd.collective_compute(
                    "AllGather", ALU.bypass,
                    replica_groups=[list(range(n_cores))],
                    ins=[zs_own_d[:]],
                    outs=[zs_all_d[:]],
                )
            else:
                nc.sync.dma_start(zs_all_d[:], zs_own_d[:])

            # ---------- phase 6: X2 build + A@X2 ----------
            with ExitStack() as p6:
                x_pool2 = p6.enter_context(tc.tile_pool(name="xsb2", bufs=1))
                X2 = x_pool2.tile([128, MC * BC], BF16)
                for j, (m0, mp) in enumerate(mch):
                    nc.sync.dma_start(
                        X2[:mp, j * BC:(j + 1) * BC]
                        .rearrange("m (b c) -> m b c", c=CAT)[:, :, :H],
                        zs_all_d[m0:m0 + mp, :, :])
                    nc.scalar.copy(
                        X2[:mp, j * BC:(j + 1) * BC]
                        .rearrange("m (b c) -> m b c", c=CAT)[:, :, H:],
                        xcols[:mp, ts(j, B * DIN)]
                        .rearrange("m (b c) -> m b c", c=DIN))
                ax_psum = p6.enter_context(tc.tile_pool(name="ax2_ps", bufs=3, space="PSUM"))
                ax_pool = p6.enter_context(tc.tile_pool(name="ax2", bufs=3))
                for q, (c0, cp) in enumerate(bcch):
                    ps = ax_psum.tile([128, NOWN], F32, tag="ax2_ps")
                    for j, (m0, mp) in enumerate(mch):
                        nc.tensor.matmul(ps[:cp, :],
                                         X2[:mp, j * BC + c0: j * BC + c0 + cp],
                                         M_sb[:mp, ts(j, NOWN)],
                                         start=(j == 0), stop=(j == MC - 1))
                    ev = ax_pool.tile([128, NOWN], BF16, tag="ax2_ev")
                    nc.vector.tensor_tensor(ev[:cp, :], ps[:cp, :],
                                            rinv_bc[:cp, :], ALU.mult)
                    nc.sync.dma_start(xg2_d[c0:c0 + cp, :], ev[:cp, :])

            # ---------- phase 7: apply update; blend; output ----------
            with ExitStack() as p7:
                xg2T_pool = p7.enter_context(tc.tile_pool(name="xg2T", bufs=1))
                xg2T = xg2T_pool.tile([CAT, B * NOWN], BF16)
                nc.sync.dma_start(
                    xg2T[:].rearrange("c (b n) -> c b n", b=B),
                    xg2_d[:].rearrange("(b c) n -> c b n", b=B))
                ap_w = p7.enter_context(tc.tile_pool(name="ap_w2", bufs=2))
                ap_ps = p7.enter_context(tc.tile_pool(name="ap_ps2", bufs=4, space="PSUM"))
                ap_ev = p7.enter_context(tc.tile_pool(name="ap_ev2", bufs=3))
                st_w = p7.enter_context(tc.tile_pool(name="st_w2", bufs=2))
                out_w = p7.enter_context(tc.tile_pool(name="out_w", bufs=3))
                for (nb0, nbp) in _chunks(NOWN, nblk):
                    w0 = ap_w.tile([KI0, nblk * OU], BF16, tag="w0u")
                    nc.sync.dma_start(
                        w0[:, :nbp * OU].rearrange("k (n o) -> k n o", o=OU),
                        wu_d[nb0:nb0 + nbp, :KI0, :].rearrange("n k o -> k n o"))
                    w1 = ap_w.tile([CAT, nblk * OU], BF16, tag="w1u")
                    nc.sync.dma_start(
                        w1[:, :nbp * OU].rearrange("k (n o) -> k n o", o=OU),
                        wu_d[nb0:nb0 + nbp, KI0:, :].rearrange("n k o -> k n o"))
                    stw = st_w.tile([B, nblk * H], F32, tag="stw2")
                    nc.sync.dma_start(
                        stw[:, :nbp * H],
                        state_own_d[:, nb0:nb0 + nbp, :]
                        .rearrange("b n h -> b (n h)"))
                    rw = st_w.tile([B, nblk * H], BF16, tag="rw")
                    nc.sync.dma_start(
                        rw[:, :nbp * H].rearrange("b (n h) -> b n h", h=H),
                        r_d[nb0:nb0 + nbp, :, :].rearrange("n b h -> b n h"))
                    hc_blk = out_w.tile([B, nblk * H], F32, tag="hc")
                    for nl in range(nbp):
                        n = nb0 + nl
                        ps = ap_ps.tile([B, OU], F32, tag="ap_ps2")
                        nc.tensor.matmul(
                            ps[:, :],
                            XT[:, n::NOWN][:, :B],
                            w0[:, nl * OU:(nl + 1) * OU],
                            start=True, stop=False)
                        nc.tensor.matmul(
                            ps[:, :],
                            xg2T[:, n::NOWN][:, :B],
                            w1[:, nl * OU:(nl + 1) * OU],
                            start=False, stop=True)
                        nc.scalar.activation(
                            hc_blk[:, nl * H:(nl + 1) * H], ps[:, :], AF.Tanh)
                    # blend whole block: out = hc + r*(state - hc)
                    t1 = out_w.tile([B, nblk * H], F32, tag="t1")
                    nc.vector.tensor_sub(t1[:, :nbp * H], stw[:, :nbp * H],
                                         hc_blk[:, :nbp * H])
                    t2 = out_w.tile([B, nblk * H], F32, tag="t2")
                    nc.vector.tensor_tensor(
                        t2[:, :nbp * H], t1[:, :nbp * H],
                        rw[:, :nbp * H], ALU.mult)
                    ot = out_w.tile([B, nblk * H], F32, tag="ot")
                    nc.vector.tensor_add(ot[:, :nbp * H], t2[:, :nbp * H],
                                         hc_blk[:, :nbp * H])
                    nc.sync.dma_start(
                        out_d[:, nb0:nb0 + nbp, :]
                        .rearrange("b n h -> b (n h)"),
                        ot[:, :nbp * H])

    nc.compile()
    return nc


_NC_CACHE = {}


def _get_nc(N, n_cores):
    key = (N, n_cores)
    if key not in _NC_CACHE:
        _NC_CACHE[key] = build_nc(N=N, n_cores=n_cores)
    return _NC_CACHE[key]


def _pack_pool(wp, bias, O):
    """[E,K,CAT,O] pool + [E,O] bias -> [E, 133, O] bf16.

    Row order matches the on-device layouts: each k-slab is (state rows,
    x rows); the bias sits at row 66 paired with the ones row of XT."""
    out = np.empty((E, KIALL, O), np.float32)
    out[:, :H, :] = wp[:, 0, DIN:, :]
    out[:, H:CAT, :] = wp[:, 0, :DIN, :]
    out[:, CAT, :] = bias
    out[:, KI0:KI0 + H, :] = wp[:, 1, DIN:, :]
    out[:, KI0 + H:, :] = wp[:, 1, :DIN, :]
    return out.astype(ml_dtypes.bfloat16)


def kernel(x, state, node_embed, gate_weights_pool, gate_bias_pool,
           update_weights_pool, update_bias_pool, n_cores=8):
    x = np.asarray(x, np.float32)
    state = np.asarray(state, np.float32)
    node_embed = np.asarray(node_embed, np.float32)
    N = node_embed.shape[0]
    NOWN = N // n_cores
    nc = _get_nc(N, n_cores)

    eT = np.ascontiguousarray(node_embed.T)                 # [E, N]
    # X1[m, (b, c)] with c = (state:64, x:2), bf16
    x1_nbc = np.concatenate([state.transpose(1, 0, 2), x.transpose(1, 0, 2)],
                            axis=2)                          # [N, B, CAT]
    x1 = x1_nbc.astype(ml_dtypes.bfloat16).reshape(N, BC)
    wpg = _pack_pool(np.asarray(gate_weights_pool, np.float32),
                     np.asarray(gate_bias_pool, np.float32), OG)
    wpu = _pack_pool(np.asarray(update_weights_pool, np.float32),
                     np.asarray(update_bias_pool, np.float32), OU)

    in_maps = []
    for c in range(n_cores):
        sl = slice(c * NOWN, (c + 1) * NOWN)
        eT_own = np.ascontiguousarray(eT[:, sl])
        xt = np.ones((KI0, B, NOWN), np.float32)
        xt[:CAT] = x1_nbc[sl].transpose(2, 1, 0)
        in_maps.append({
            "x1_in": x1,
            "xt_in": xt.astype(ml_dtypes.bfloat16).reshape(KI0, B * NOWN),
            "state_own": np.ascontiguousarray(state[:, sl, :]),
            "eT_all": eT,
            "eT_own": eT_own,
            "eT_own_bf": eT_own.astype(ml_dtypes.bfloat16),
            "wpg": wpg,
            "wpu": wpu,
        })
    res = run_bass_kernel_spmd(nc, in_maps, list(range(n_cores)))
    out = np.concatenate([res.results[c]["out"] for c in range(n_cores)], axis=1)
    return out.astype(np.float32)

